# revision 74
# baseline (speedup 1.0000x reference)
"""Trainium2 Bass kernel for nn_CapsuleLayer (capsule dynamic routing).

Math (reference):
    u_hat[b,c,u,s] = sum_i W[c,u,s,i] * x[b,i,c]          (never materialized)
    3 routing iterations:
        c_ij = softmax_u(b_ij)                            [C, U]
        s_j[b,u,s]  = sum_c c_ij[c,u] * u_hat[b,c,u,s]
        v_j = squash(s_j)   (norm over the U axis)
        u_vj1[c,u] = sum_{b,s} u_hat[b,c,u,s] v_j[b,u,s] / B
        b_ij += u_vj1
    output = v_j  (B, U, S, 1)

Sharding: channels C=1152 split 8 ways (CL=144 per core).  Per core both
x-slice and W-slice live in SBUF; u_hat is recomputed on the fly as matrix
products (contraction over (c,i)=2304 or over b=256).  One 160 KB f16
AllReduce of the s_j partial per routing iteration (ReduceScatter on the
last).

Key performance structure vs the naive version:
  * W is staged pre-scaled by 1/10 so iteration 0 needs no extra scale
    (c_ij0 = 1/10); later iterations use c_ij*10, and the block-diagonal
    mean matrix EM carries a compensating x10.
  * Act uses only {Copy, Sqrt, Exp}.  The sqrt<->exp table switches are
    placed where Act is idle (the sqrt load hides inside the collective
    window, the exp load before softmax's first exp), so no activation
    table load sits on the serial critical path.
  * softmax skips the max-subtraction (b_ij in [-4, 16], f32 exp is safe).
  * The agreement pipeline is grouped (G=3): q/r/softmax/bm/mm1 for group
    g run while group g+1 is still in matmul, and engine assignment spreads
    work across Act (PSUM->SBUF copies), Pool (q multiply), DVE (reduce,
    softmax, bm, squash) and PE (matmuls).
  * Dummy "junk" matmuls (own PSUM bank, dedicated source tile) keep the
    tensor engine busy during each AllReduce: the PE p-state ramp otherwise
    drops the clock from 2.4 GHz to 1.2 GHz after every stall.

Per-core layouts (host-prepared), column convention col = s*10+u:
    XT [128, T*B ] f16 : XT[p, t*256+b]      = x[b, i, c],    ci = 128t+p
    XF [128, 2*KCI] f16: XF[p, bc*2304+ci]   = x[b, i, c],    b  = 128bc+p
    WM [128, T*US] f16 : WM[p, t*320+s*10+u] = W[c,u,s,i]/10, ci = 128t+p
    EM [128, 128] f16  : block-diag, EM[p,m] = (p//16==m//16) * 10/256
    OUT [16, 2*US] f16 : this core's ReduceScatter shard of s_j
                         (the final squash runs on the host)
"""

import numpy as np

B, IN_U, C, NUM_U, S = 256, 16, 1152, 10, 32
NCORES = 8
CL = C // NCORES          # 144 channels per core
KCI = CL * IN_U           # 2304 contraction size
T = KCI // 128            # 18 partition chunks
US = NUM_U * S            # 320
NITER = 3
G = 3                     # chunk groups (pipelining granularity)
CPG = T // G              # 6 chunks per group
PR = 128 // NCORES        # 16 partition rows per core after ReduceScatter

JUNK_N = 512              # free size of one junk matmul (f32: 2048 PE cycles)
K_AR = 50                 # junk matmuls covering one AllReduce + squash
K_PRE = 1                 # junk matmuls bridging DMA gaps in the preamble
K_MID = (12, 5, 4)        # f16 junk bridges inside the agreement phase
N_DEFER = 0               # mm2 PSUM->SBUF copies deferred from Act to DVE
N_FRONT = 2               # leading mm2 copies taken by DVE in its idle bubble
DUMMY_EXP_AT = 11         # mm2 chunk after which the exp-table reload is pulled in
GROUP_SPLIT = (6, 6, 6)   # chunks per pipeline group (sums to T=18)
LOAD_SPANS = ((0, 6), (6, 10), (10, 14), (14, 16), (16, 18))  # preamble load granularity

_CACHE = {}


def _build_program(bypass_cc=False):
    import concourse.bacc as bacc
    import concourse.tile as tile
    from concourse import mybir
    from contextlib import ExitStack

    f32 = mybir.dt.float32
    f16 = mybir.dt.float16
    AX = mybir.AxisListType
    ALU = mybir.AluOpType
    AF = mybir.ActivationFunctionType

    nc = bacc.Bacc(None, num_devices=NCORES)
    xt_d = nc.declare_dram_parameter("xt", [128, T * B], f16, isOutput=False)
    xf_d = nc.declare_dram_parameter("xf", [128, 2 * KCI], f16, isOutput=False)
    wm_d = nc.declare_dram_parameter("wm", [128, T * US], f16, isOutput=False)
    em_d = nc.declare_dram_parameter("em", [128, 128], f16, isOutput=False)
    # out = this core's ReduceScatter shard of s_j (pre-squash, f16); the
    # final squash runs on the host in postprocess().
    out_d = nc.declare_dram_parameter("out", [PR, 2 * US], f16, isOutput=True)

    with tile.TileContext(nc) as tc, ExitStack() as ctx:
        singles = ctx.enter_context(tc.tile_pool(name="singles", bufs=1))
        big = ctx.enter_context(tc.tile_pool(name="big", bufs=2))
        work = ctx.enter_context(tc.tile_pool(name="work", bufs=2))
        psum_s = ctx.enter_context(tc.tile_pool(name="psum_s", bufs=1, space="PSUM"))
        psum_m = ctx.enter_context(tc.tile_pool(name="psum_m", bufs=3, space="PSUM"))
        psum_u = ctx.enter_context(tc.tile_pool(name="psum_u", bufs=1, space="PSUM"))
        psum_j = ctx.enter_context(tc.tile_pool(name="psum_j", bufs=2, space="PSUM"))
        dram = ctx.enter_context(tc.tile_pool(name="dram", bufs=2, space="DRAM"))

        # --- resident tiles -------------------------------------------------
        em_sb = singles.tile([128, 128], f16, name="em_sb")
        nc.sync.dma_start(out=em_sb, in_=em_d[:])
        jt = singles.tile([128, JUNK_N], f32, name="jt")
        nc.gpsimd.memset(jt, 0.0)
        bij_sb = singles.tile([128, T * NUM_U], f32, name="bij_sb")
        nc.vector.memset(bij_sb, 0.0)
        # Dummy Exp: hoists the initial activation-table load off the
        # critical path (runs during the input DMAs).
        actwarm = singles.tile([128, 2], f32, name="actwarm")
        nc.scalar.activation(out=actwarm, in_=em_sb[:, 0:2], func=AF.Exp)

        xt_sb = singles.tile([128, T * B], f16, name="xt_sb")
        wm_sb = singles.tile([128, T * US], f16, name="wm_sb")
        # the last group loads in two half-pairs: mm1's tail then waits on a
        # 3-chunk transfer + sem instead of a 6-chunk one (~0.8us earlier AR0)
        spans = LOAD_SPANS
        for lo, hi in spans:
            nc.sync.dma_start(
                out=xt_sb[:, lo * B : hi * B], in_=xt_d[:, lo * B : hi * B]
            )
            nc.sync.dma_start(
                out=wm_sb[:, lo * US : hi * US], in_=wm_d[:, lo * US : hi * US]
            )
        # xf is first needed by mm2 (after AllReduce 0): its loads are emitted
        # after the first collective so they don't contend with the
        # xt/wm preamble loads or the cc staging DMA.
        xf_sb = singles.tile([128, 2 * KCI], f16, name="xf_sb")

        def load_xf(gate):
            # Corner-write creates a WAW dep: the xf DMAs (which would
            # otherwise bypass via the SP wait queue and hog DMA_ENGINES
            # while the cc stage transfer is ready) start only after `gate`.
            nc.vector.tensor_copy(out=xf_sb[0:1, 0:2], in_=gate[0:1, 0:2])
            for bc in range(2):
                nc.sync.dma_start(
                    out=xf_sb[:, bc * KCI : (bc + 1) * KCI],
                    in_=xf_d[:, bc * KCI : (bc + 1) * KCI],
                )

        def emit_junk(n, gate=None):
            """Matmuls that keep the PE p-state ramped during stalls.  With
            `gate`, each reads the gate tile so none can run before it is
            written (keeps them inside the intended stall window)."""
            rhs = jt[:] if gate is None else gate[:]
            for _ in range(n):
                psj = psum_j.tile([128, JUNK_N], f32, name="psj")
                nc.tensor.matmul(
                    psj, lhsT=jt[:, 0:128], rhs=rhs, start=True, stop=True
                )

        def emit_junk_f16(n, gate):
            """Short f16 junk matmuls (213 ns each) bridging PE idle gaps
            inside the agreement phase so the p-state ramp survives."""
            for _ in range(n):
                psj = psum_j.tile([128, JUNK_N], f32, name="psj")
                # gate on the LAST columns: tile deps are slice-level, so
                # reading the tail means "after the whole tile is written"
                w = gate.shape[1]
                nc.tensor.matmul(
                    psj, lhsT=wm_sb[:, 0:128], rhs=gate[:, w - JUNK_N : w],
                    start=True, stop=True,
                )

        def stage_and_reduce(cc_sb, last):
            """SBUF -> DRAM -> collective (-> DRAM -> SBUF unless last, where
            the ReduceScatter writes the output tensor directly).  The
            stage-in runs as two half DMAs so the first starts (descgen and
            all) as soon as the bc0 PSUM->SBUF copy lands."""
            cc_in = dram.tile([128, 2 * US], f16, name="cc_in")
            nc.sync.dma_start(out=cc_in, in_=cc_sb)
            if last:
                rs_out = dram.tile([PR, 2 * US], f16, name="rs_out")
                if bypass_cc:
                    nc.gpsimd.dma_start(out=rs_out, in_=cc_in[0:PR, :])
                else:
                    nc.gpsimd.collective_compute(
                        "ReduceScatter",
                        ALU.add,
                        replica_groups=[list(range(NCORES))],
                        ins=[cc_in.opt()],
                        outs=[rs_out.opt()],
                    )
                nc.sync.dma_start(out=out_d[:], in_=rs_out)
                return None
            cc_out = dram.tile([128, 2 * US], f16, name="cc_out")
            if bypass_cc:
                nc.gpsimd.dma_start(out=cc_out, in_=cc_in)
            else:
                nc.gpsimd.collective_compute(
                    "AllReduce",
                    ALU.add,
                    replica_groups=[list(range(NCORES))],
                    ins=[cc_in.opt()],
                    outs=[cc_out.opt()],
                )
            s_sb = work.tile([128, 2 * US], f16, name="s16")
            for bc in range(2):
                h = slice(bc * US, (bc + 1) * US)
                nc.sync.dma_start(out=s_sb[:, h], in_=cc_out[:, h])
            return s_sb

        def squash(s_sb, rows=128, out_dt=f16):
            """v = s * mag/(1+mag^2); mag^2 summed over u per (b, s').
            Emitted per bc-half: the bc0 chain starts as soon as the first
            copyback half lands, and mm2's first (bc0) matmuls need only
            v_bf[:, :US]."""
            v_sb = work.tile([rows, 2 * US], out_dt, name="v_sb")
            for bc in range(2):
                h = slice(bc * US, (bc + 1) * US)
                hs = slice(bc * S, (bc + 1) * S)
                sq = work.tile([rows, US], f16, name="sq")
                nc.vector.tensor_mul(out=sq, in0=s_sb[:, h], in1=s_sb[:, h])
                magsq = work.tile([rows, S], f32, name="magsq")
                nc.vector.reduce_sum(
                    out=magsq,
                    in_=sq.rearrange("p (s u) -> p s u", s=S),
                    axis=AX.X,
                )
                # Act Sqrt: its table load hides in the preceding collective
                # window (the load has no data deps and Act is idle there);
                # the exp-table reload then lands before softmax's first exp,
                # where Act is also idle - neither load sits on this path.
                mag = work.tile([rows, S], f32, name="mag")
                nc.scalar.sqrt(out=mag, in_=magsq)
                den = work.tile([rows, S], f32, name="den")
                nc.vector.tensor_scalar_add(out=den, in0=magsq, scalar1=1.0)
                rden = work.tile([rows, S], f32, name="rden")
                nc.vector.reciprocal(out=rden, in_=den)
                fct = work.tile([rows, S], f32, name="fct")
                nc.vector.tensor_mul(out=fct, in0=mag, in1=rden)
                nc.vector.tensor_mul(
                    out=v_sb[:, h].rearrange("p (s u) -> p s u", s=S),
                    in0=s_sb[:, h].rearrange("p (s u) -> p s u", s=S),
                    in1=fct.unsqueeze(2).broadcast_to([rows, S, NUM_U]),
                )
            return v_sb

        # --- iteration 0: uniform routing, mm1 rhs = wm (pre-scaled 1/10) ---
        ps_cc = [psum_s.tile([128, US], f32, name=f"s_ps{bc}") for bc in range(2)]
        for t in range(T):
            g, tl = divmod(t, CPG)
            for bc in range(2):
                nc.tensor.matmul(
                    ps_cc[bc],
                    lhsT=xt_sb[:, t * B + bc * 128 : t * B + bc * 128 + 128],
                    rhs=wm_sb[:, t * US : (t + 1) * US],
                    start=(t == 0),
                    stop=(t == T - 1),
                )
            if tl == CPG - 1 and g < G - 1:
                emit_junk(K_PRE)  # bridge DMA-bound gaps between groups
        cc_sb = work.tile([128, 2 * US], f16, name="cc_sb")
        nc.scalar.copy(out=cc_sb[:, 0:US], in_=ps_cc[0])
        nc.vector.tensor_copy(out=cc_sb[:, US : 2 * US], in_=ps_cc[1])
        jt2 = work.tile([128, JUNK_N], f32, name="jt2")
        nc.vector.tensor_copy(out=jt2, in_=cc_sb[:, 0:JUNK_N])
        emit_junk(K_AR, gate=jt2)
        s_sb = stage_and_reduce(cc_sb, last=False)
        load_xf(jt2)

        # --- iterations 1, 2 ------------------------------------------------
        for it in range(1, NITER):
            v_bf = squash(s_sb)

            # mm2: m[ci, su] = sum_b x[b,ci] v[b,su].  PSUM -> SBUF f16 copies
            # go to Act for the first 14 chunks; the last 4 are deferred to
            # DVE's idle slot after the group-0 tree, so the Act queue (in
            # order!) reaches softmax's exp before all 18 copies drain.
            m16 = big.tile([128, T * US], f16, name="m16")
            deferred = []
            for t in range(T):
                ps = psum_m.tile([128, US], f32, name="m_ps")
                for bc in range(2):
                    nc.tensor.matmul(
                        ps,
                        lhsT=xf_sb[:, bc * KCI + t * 128 : bc * KCI + (t + 1) * 128],
                        rhs=v_bf[:, bc * US : (bc + 1) * US],
                        start=(bc == 0),
                        stop=(bc == 1),
                    )
                if t < N_FRONT:
                    # DVE is idle between squash and q0: it absorbs the
                    # first copies, so m16-g0 completes sooner and Act's
                    # queue reaches softmax exp earlier
                    nc.vector.tensor_copy(out=m16[:, t * US : (t + 1) * US], in_=ps)
                elif t < T - N_DEFER:
                    nc.scalar.copy(out=m16[:, t * US : (t + 1) * US], in_=ps)
                else:
                    deferred.append((t, ps))
                if t == DUMMY_EXP_AT:
                    # dummy exp: pulls the sqrt->exp act-table reload to this
                    # point in the Act queue, off the softmax-g0 chain
                    nc.scalar.activation(
                        out=actwarm, in_=em_sb[:, 0:2], func=AF.Exp
                    )

            ps_cc = [psum_s.tile([128, US], f32, name=f"s_ps{bc}") for bc in range(2)]
            ups = psum_u.tile([128, T * NUM_U], f32, name="u_ps")
            r_sb = work.tile([128, T * NUM_U], f16, name="r_sb")
            ex = work.tile([128, T * NUM_U], f32, name="ex")
            sm = work.tile([128, T], f32, name="sm")
            rsm = work.tile([128, T], f32, name="rsm")
            cij = work.tile([128, T * NUM_U], f16, name="cij")

            # Uneven groups: a small last group shortens the serial tail
            # (q2/tree2/bm2/mm1-g2 gate the cc staging before the collective)
            GS = list(GROUP_SPLIT)
            OFF = [sum(GS[:i]) for i in range(len(GS))]

            def colS(g):
                return slice(OFF[g] * US, (OFF[g] + GS[g]) * US)

            # q = wm * m.  Emission order places q0 and q2 on DVE around the
            # group-0 tree/softmax/bm work; q1 runs on Pool in parallel.
            q_t = [
                big.tile([128, GS[g] * US], f16, name=f"q_{g}") for g in range(G)
            ]

            def emit_q(g):
                if g >= 1:
                    # split: Pool's 0.42-efficiency multiply would gate the
                    # group-1 tree for 3.9us; giving half to DVE pulls the
                    # gate in by ~2us while costing DVE only ~0.5us
                    hn = GS[g] // 2
                    lo = slice(OFF[g] * US, (OFF[g] + hn) * US)
                    hi = slice((OFF[g] + hn) * US, (OFF[g] + GS[g]) * US)
                    nc.gpsimd.tensor_mul(
                        out=q_t[g][:, 0 : hn * US], in0=wm_sb[:, lo], in1=m16[:, lo]
                    )
                    nc.vector.tensor_mul(
                        out=q_t[g][:, hn * US :], in0=wm_sb[:, hi], in1=m16[:, hi]
                    )
                else:
                    nc.vector.tensor_mul(
                        out=q_t[g], in0=wm_sb[:, colS(g)], in1=m16[:, colS(g)]
                    )

            emit_q(0)
            emit_q(1)
            for g in range(G):
                ng = GS[g]
                cS = colS(g)
                uS = slice(OFF[g] * NUM_U, (OFF[g] + ng) * NUM_U)  # u cols
                tS = slice(OFF[g], OFF[g] + ng)
                # r = sum_s q via pairwise-halving adds (TensorTensor runs at
                # 2x for packed f16; TensorReduce is always 1x, so shrink its
                # input first): 32 -> 16 -> 8 s-terms, then reduce.
                qv = q_t[g].rearrange("p (t s u) -> p t s u", t=ng, s=S)
                h1 = big.tile([128, ng * (S // 2) * NUM_U], f16, name="h1")
                h1v = h1.rearrange("p (t s u) -> p t s u", t=ng, s=S // 2)
                nc.vector.tensor_add(
                    out=h1v, in0=qv[:, :, 0 : S // 2, :], in1=qv[:, :, S // 2 : S, :]
                )
                h2 = big.tile([128, ng * (S // 4) * NUM_U], f16, name="h2")
                h2v = h2.rearrange("p (t s u) -> p t s u", t=ng, s=S // 4)
                nc.vector.tensor_add(
                    out=h2v,
                    in0=h1v[:, :, 0 : S // 4, :],
                    in1=h1v[:, :, S // 4 : S // 2, :],
                )
                with nc.allow_low_precision(reason="r in f16: |r|<400, rel 5e-4"):
                    nc.vector.reduce_sum(
                        out=r_sb[:, uS],
                        in_=h2v.transpose([0, 1, 3, 2]),
                        axis=AX.X,
                    )
                # b_ij += EM @ r  (EM carries x10/256); softmax without max-sub
                nc.tensor.matmul(
                    ups[:, uS], lhsT=em_sb, rhs=r_sb[:, uS], start=True, stop=True
                )
                if g == 0:
                    emit_junk_f16(K_MID[0], q_t[0])
                nc.vector.tensor_add(
                    out=bij_sb[:, uS], in0=bij_sb[:, uS], in1=ups[:, uS]
                )
                nc.scalar.activation(out=ex[:, uS], in_=bij_sb[:, uS], func=AF.Exp)
                nc.vector.reduce_sum(
                    out=sm[:, tS],
                    in_=ex[:, uS].rearrange("p (t u) -> p t u", t=ng),
                    axis=AX.X,
                )
                # rsm = 10/sm  (x10 compensates the 1/10 baked into WM)
                nc.vector.tensor_scalar_mul(out=rsm[:, tS], in0=sm[:, tS], scalar1=0.1)
                nc.vector.reciprocal(out=rsm[:, tS], in_=rsm[:, tS])
                nc.vector.tensor_mul(
                    out=cij[:, uS].rearrange("p (t u) -> p t u", t=ng),
                    in0=ex[:, uS].rearrange("p (t u) -> p t u", t=ng),
                    in1=rsm[:, tS].unsqueeze(2).broadcast_to([128, ng, NUM_U]),
                )
                bm_g = big.tile([128, ng * US], f16, name="bm_g")
                nc.vector.tensor_mul(
                    out=bm_g.rearrange("p (t s u) -> p t s u", t=ng, s=S),
                    in0=wm_sb[:, cS].rearrange("p (t s u) -> p t s u", t=ng, s=S),
                    in1=cij[:, uS]
                    .rearrange("p (t u) -> p t u", t=ng)
                    .unsqueeze(2)
                    .broadcast_to([128, ng, S, NUM_U]),
                )
                for tl in range(ng):
                    t = OFF[g] + tl
                    for bc in range(2):
                        nc.tensor.matmul(
                            ps_cc[bc],
                            lhsT=xt_sb[:, t * B + bc * 128 : t * B + bc * 128 + 128],
                            rhs=bm_g[:, tl * US : (tl + 1) * US],
                            start=(t == 0),
                            stop=(t == T - 1),
                        )
                if g == 0:
                    # deferred mm2 copies: emitted after the group-0 softmax
                    # chain so DVE prioritizes unblocking bm0/mm1-g0, then
                    # fills its idle with these PSUM reads (they gate q2)
                    for t, ps in deferred:
                        nc.vector.tensor_copy(
                            out=m16[:, t * US : (t + 1) * US], in_=ps
                        )
                    emit_q(2)
                    emit_junk_f16(K_MID[1], q_t[0])
                elif g == 1:
                    emit_junk_f16(K_MID[2], q_t[0])
            cc_sb = work.tile([128, 2 * US], f16, name="cc_sb")
            nc.scalar.copy(out=cc_sb[:, 0:US], in_=ps_cc[0])
            nc.vector.tensor_copy(out=cc_sb[:, US : 2 * US], in_=ps_cc[1])
            last = it == NITER - 1
            if not last:
                jt2 = work.tile([128, JUNK_N], f32, name="jt2")
                nc.vector.tensor_copy(out=jt2, in_=cc_sb[:, 0:JUNK_N])
                emit_junk(K_AR, gate=jt2)
            s_sb = stage_and_reduce(cc_sb, last=last)

    return nc


def _prep_core_inputs(x, W, core, em):
    sl = slice(core * CL, (core + 1) * CL)
    xs = np.ascontiguousarray(x[:, :, sl])  # (B, I, CL)
    ws = np.ascontiguousarray(W[0, sl])     # (CL, U, S, I)
    xt = xs.transpose(2, 1, 0).reshape(T, 128, B)
    xt = np.ascontiguousarray(xt.transpose(1, 0, 2)).reshape(128, T * B)
    xf = xs.transpose(0, 2, 1).reshape(2, 128, KCI)
    xf = np.ascontiguousarray(xf.transpose(1, 0, 2)).reshape(128, 2 * KCI)
    wm = (ws / float(NUM_U)).transpose(0, 3, 2, 1).reshape(T, 128, US)
    wm = np.ascontiguousarray(wm.transpose(1, 0, 2)).reshape(128, T * US)
    return {
        "xt": xt.astype(np.float16),
        "xf": xf.astype(np.float16),
        "wm": wm.astype(np.float16),
        "em": em,
    }


def prep_in_maps(x, W):
    x = np.asarray(x, dtype=np.float32)
    W = np.asarray(W, dtype=np.float32)
    em = (np.kron(np.eye(8, dtype=np.float32), np.ones((16, 16), np.float32))
          * (float(NUM_U) / float(B))).astype(np.float16)
    return [_prep_core_inputs(x, W, core, em) for core in range(NCORES)]


def postprocess(results):
    """Assemble per-core ReduceScatter shards (16 partition rows each) of
    the pre-squash s_j into [128, 640] (col = bc*320 + s*10 + u), apply the
    final squash on the host, then -> (B, U, S, 1)."""
    full = np.concatenate(
        [np.asarray(results[r]["out"], np.float64) for r in range(NCORES)],
        axis=0,
    )
    s = full.reshape(128, 2, S, NUM_U).transpose(1, 0, 3, 2)  # (bc,p,u,s)
    s = s.reshape(B, NUM_U, S)
    m2 = np.sum(s * s, axis=1, keepdims=True)  # norm over the U axis
    v = s * (np.sqrt(m2) / (1.0 + m2))
    return np.ascontiguousarray(v[..., None].astype(np.float32))


def get_program():
    if "nc" not in _CACHE:
        nc = _build_program()
        nc.finalize()
        _CACHE["nc"] = nc
    return _CACHE["nc"]


def kernel(x, W):
    from concourse.bass_utils import run_bass_kernel_spmd

    nc = get_program()
    in_maps = prep_in_maps(x, W)
    res = run_bass_kernel_spmd(nc, in_maps, list(range(NCORES)))
    return postprocess(res.results)


# revision 75
# speedup vs baseline: 1.0027x; 1.0027x over previous
"""Trainium2 Bass kernel for nn_CapsuleLayer (capsule dynamic routing).

Math (reference):
    u_hat[b,c,u,s] = sum_i W[c,u,s,i] * x[b,i,c]          (never materialized)
    3 routing iterations:
        c_ij = softmax_u(b_ij)                            [C, U]
        s_j[b,u,s]  = sum_c c_ij[c,u] * u_hat[b,c,u,s]
        v_j = squash(s_j)   (norm over the U axis)
        u_vj1[c,u] = sum_{b,s} u_hat[b,c,u,s] v_j[b,u,s] / B
        b_ij += u_vj1
    output = v_j  (B, U, S, 1)

Sharding: channels C=1152 split 8 ways (CL=144 per core).  Per core both
x-slice and W-slice live in SBUF; u_hat is recomputed on the fly as matrix
products (contraction over (c,i)=2304 or over b=256).  One 160 KB f16
AllReduce of the s_j partial per routing iteration (ReduceScatter on the
last).

Key performance structure vs the naive version:
  * W is staged pre-scaled by 1/10 so iteration 0 needs no extra scale
    (c_ij0 = 1/10); later iterations use c_ij*10, and the block-diagonal
    mean matrix EM carries a compensating x10.
  * Act uses only {Copy, Sqrt, Exp}.  The sqrt<->exp table switches are
    placed where Act is idle (the sqrt load hides inside the collective
    window, the exp load before softmax's first exp), so no activation
    table load sits on the serial critical path.
  * softmax skips the max-subtraction (b_ij in [-4, 16], f32 exp is safe).
  * The agreement pipeline is grouped (G=3): q/r/softmax/bm/mm1 for group
    g run while group g+1 is still in matmul, and engine assignment spreads
    work across Act (PSUM->SBUF copies), Pool (q multiply), DVE (reduce,
    softmax, bm, squash) and PE (matmuls).
  * Dummy "junk" matmuls (own PSUM bank, dedicated source tile) keep the
    tensor engine busy during each AllReduce: the PE p-state ramp otherwise
    drops the clock from 2.4 GHz to 1.2 GHz after every stall.

Per-core layouts (host-prepared), column convention col = s*10+u:
    XT [128, T*B ] f16 : XT[p, t*256+b]      = x[b, i, c],    ci = 128t+p
    XF [128, 2*KCI] f16: XF[p, bc*2304+ci]   = x[b, i, c],    b  = 128bc+p
    WM [128, T*US] f16 : WM[p, t*320+s*10+u] = W[c,u,s,i]/10, ci = 128t+p
    EM [128, 128] f16  : block-diag, EM[p,m] = (p//16==m//16) * 10/256
    OUT [16, 2*US] f16 : this core's ReduceScatter shard of s_j
                         (the final squash runs on the host)
"""

import numpy as np

B, IN_U, C, NUM_U, S = 256, 16, 1152, 10, 32
NCORES = 8
CL = C // NCORES          # 144 channels per core
KCI = CL * IN_U           # 2304 contraction size
T = KCI // 128            # 18 partition chunks
US = NUM_U * S            # 320
NITER = 3
G = 3                     # chunk groups (pipelining granularity)
CPG = T // G              # 6 chunks per group
PR = 128 // NCORES        # 16 partition rows per core after ReduceScatter

JUNK_N = 512              # free size of one junk matmul (f32: 2048 PE cycles)
K_AR = 50                 # junk matmuls covering one AllReduce + squash
K_PRE = 1                 # junk matmuls bridging DMA gaps in the preamble
K_MID = (12, 5, 4)        # f16 junk bridges inside the agreement phase
N_DEFER = 0               # mm2 PSUM->SBUF copies deferred from Act to DVE
N_FRONT = 2               # leading mm2 copies taken by DVE in its idle bubble
DUMMY_EXP_AT = 11         # mm2 chunk after which the exp-table reload is pulled in
GROUP_SPLIT = (6, 6, 6)   # chunks per pipeline group (sums to T=18)
LOAD_SPANS = ((0, 6), (6, 10), (10, 14), (14, 16), (16, 18))  # preamble load granularity

_CACHE = {}


def _build_program(bypass_cc=False):
    import concourse.bacc as bacc
    import concourse.tile as tile
    from concourse import mybir
    from contextlib import ExitStack

    f32 = mybir.dt.float32
    f16 = mybir.dt.float16
    AX = mybir.AxisListType
    ALU = mybir.AluOpType
    AF = mybir.ActivationFunctionType

    nc = bacc.Bacc(None, num_devices=NCORES)
    xt_d = nc.declare_dram_parameter("xt", [128, T * B], f16, isOutput=False)
    xf_d = nc.declare_dram_parameter("xf", [128, 2 * KCI], f16, isOutput=False)
    wm_d = nc.declare_dram_parameter("wm", [128, T * US], f16, isOutput=False)
    em_d = nc.declare_dram_parameter("em", [128, 128], f16, isOutput=False)
    # out = this core's ReduceScatter shard of s_j (pre-squash, f16); the
    # final squash runs on the host in postprocess().
    out_d = nc.declare_dram_parameter("out", [PR, 2 * US], f16, isOutput=True)

    with tile.TileContext(nc) as tc, ExitStack() as ctx:
        singles = ctx.enter_context(tc.tile_pool(name="singles", bufs=1))
        big = ctx.enter_context(tc.tile_pool(name="big", bufs=2))
        work = ctx.enter_context(tc.tile_pool(name="work", bufs=2))
        psum_s = ctx.enter_context(tc.tile_pool(name="psum_s", bufs=1, space="PSUM"))
        psum_m = ctx.enter_context(tc.tile_pool(name="psum_m", bufs=3, space="PSUM"))
        psum_u = ctx.enter_context(tc.tile_pool(name="psum_u", bufs=1, space="PSUM"))
        psum_j = ctx.enter_context(tc.tile_pool(name="psum_j", bufs=2, space="PSUM"))
        dram = ctx.enter_context(tc.tile_pool(name="dram", bufs=2, space="DRAM"))

        # --- resident tiles -------------------------------------------------
        em_sb = singles.tile([128, 128], f16, name="em_sb")
        nc.sync.dma_start(out=em_sb, in_=em_d[:])
        jt = singles.tile([128, JUNK_N], f32, name="jt")
        nc.gpsimd.memset(jt, 0.0)
        bij_sb = singles.tile([128, T * NUM_U], f32, name="bij_sb")
        nc.vector.memset(bij_sb, 0.0)
        # Dummy Exp: hoists the initial activation-table load off the
        # critical path (runs during the input DMAs).
        actwarm = singles.tile([128, 2], f32, name="actwarm")
        nc.scalar.activation(out=actwarm, in_=em_sb[:, 0:2], func=AF.Exp)

        xt_sb = singles.tile([128, T * B], f16, name="xt_sb")
        wm_sb = singles.tile([128, T * US], f16, name="wm_sb")
        # the last group loads in two half-pairs: mm1's tail then waits on a
        # 3-chunk transfer + sem instead of a 6-chunk one (~0.8us earlier AR0)
        spans = LOAD_SPANS
        for lo, hi in spans:
            nc.sync.dma_start(
                out=xt_sb[:, lo * B : hi * B], in_=xt_d[:, lo * B : hi * B]
            )
            nc.sync.dma_start(
                out=wm_sb[:, lo * US : hi * US], in_=wm_d[:, lo * US : hi * US]
            )
        # xf is first needed by mm2 (after AllReduce 0): its loads are emitted
        # after the first collective so they don't contend with the
        # xt/wm preamble loads or the cc staging DMA.
        xf_sb = singles.tile([128, 2 * KCI], f16, name="xf_sb")

        def load_xf(gate):
            # Corner-write creates a WAW dep: the xf DMAs (which would
            # otherwise bypass via the SP wait queue and hog DMA_ENGINES
            # while the cc stage transfer is ready) start only after `gate`.
            nc.vector.tensor_copy(out=xf_sb[0:1, 0:2], in_=gate[0:1, 0:2])
            for bc in range(2):
                nc.sync.dma_start(
                    out=xf_sb[:, bc * KCI : (bc + 1) * KCI],
                    in_=xf_d[:, bc * KCI : (bc + 1) * KCI],
                )

        def emit_junk(n, gate=None):
            """Matmuls that keep the PE p-state ramped during stalls.  With
            `gate`, each reads the gate tile so none can run before it is
            written (keeps them inside the intended stall window)."""
            rhs = jt[:] if gate is None else gate[:]
            for _ in range(n):
                psj = psum_j.tile([128, JUNK_N], f32, name="psj")
                nc.tensor.matmul(
                    psj, lhsT=jt[:, 0:128], rhs=rhs, start=True, stop=True
                )

        def emit_junk_f16(n, gate):
            """Short f16 junk matmuls (213 ns each) bridging PE idle gaps
            inside the agreement phase so the p-state ramp survives."""
            for _ in range(n):
                psj = psum_j.tile([128, JUNK_N], f32, name="psj")
                # gate on the LAST columns: tile deps are slice-level, so
                # reading the tail means "after the whole tile is written"
                w = gate.shape[1]
                nc.tensor.matmul(
                    psj, lhsT=wm_sb[:, 0:128], rhs=gate[:, w - JUNK_N : w],
                    start=True, stop=True,
                )

        def stage_and_reduce(cc_sb, last):
            """SBUF -> DRAM -> collective (-> DRAM -> SBUF unless last, where
            the ReduceScatter writes the output tensor directly).  The
            stage-in runs as two half DMAs so the first starts (descgen and
            all) as soon as the bc0 PSUM->SBUF copy lands."""
            cc_in = dram.tile([128, 2 * US], f16, name="cc_in")
            nc.sync.dma_start(out=cc_in, in_=cc_sb)
            if last:
                rs_out = dram.tile([PR, 2 * US], f16, name="rs_out")
                if bypass_cc:
                    nc.gpsimd.dma_start(out=rs_out, in_=cc_in[0:PR, :])
                else:
                    nc.gpsimd.collective_compute(
                        "ReduceScatter",
                        ALU.add,
                        replica_groups=[list(range(NCORES))],
                        ins=[cc_in.opt()],
                        outs=[rs_out.opt()],
                    )
                nc.sync.dma_start(out=out_d[:], in_=rs_out)
                return None
            cc_out = dram.tile([128, 2 * US], f16, name="cc_out")
            if bypass_cc:
                nc.gpsimd.dma_start(out=cc_out, in_=cc_in)
            else:
                nc.gpsimd.collective_compute(
                    "AllReduce",
                    ALU.add,
                    replica_groups=[list(range(NCORES))],
                    ins=[cc_in.opt()],
                    outs=[cc_out.opt()],
                )
            s_sb = work.tile([128, 2 * US], f16, name="s16")
            for bc in range(2):
                h = slice(bc * US, (bc + 1) * US)
                nc.sync.dma_start(out=s_sb[:, h], in_=cc_out[:, h])
            return s_sb

        def squash(s_sb, rows=128, out_dt=f16):
            """v = s * mag/(1+mag^2); mag^2 summed over u per (b, s').
            Emitted per bc-half: the bc0 chain starts as soon as the first
            copyback half lands, and mm2's first (bc0) matmuls need only
            v_bf[:, :US]."""
            v_sb = work.tile([rows, 2 * US], out_dt, name="v_sb")
            for bc in range(2):
                h = slice(bc * US, (bc + 1) * US)
                hs = slice(bc * S, (bc + 1) * S)
                sq = work.tile([rows, US], f16, name="sq")
                nc.vector.tensor_mul(out=sq, in0=s_sb[:, h], in1=s_sb[:, h])
                magsq = work.tile([rows, S], f32, name="magsq")
                nc.vector.reduce_sum(
                    out=magsq,
                    in_=sq.rearrange("p (s u) -> p s u", s=S),
                    axis=AX.X,
                )
                # Act Sqrt: its table load hides in the preceding collective
                # window (the load has no data deps and Act is idle there);
                # the exp-table reload then lands before softmax's first exp,
                # where Act is also idle - neither load sits on this path.
                mag = work.tile([rows, S], f32, name="mag")
                nc.scalar.sqrt(out=mag, in_=magsq)
                den = work.tile([rows, S], f32, name="den")
                nc.vector.tensor_scalar_add(out=den, in0=magsq, scalar1=1.0)
                rden = work.tile([rows, S], f32, name="rden")
                nc.vector.reciprocal(out=rden, in_=den)
                fct = work.tile([rows, S], f32, name="fct")
                nc.vector.tensor_mul(out=fct, in0=mag, in1=rden)
                nc.vector.tensor_mul(
                    out=v_sb[:, h].rearrange("p (s u) -> p s u", s=S),
                    in0=s_sb[:, h].rearrange("p (s u) -> p s u", s=S),
                    in1=fct.unsqueeze(2).broadcast_to([rows, S, NUM_U]),
                )
            return v_sb

        # --- iteration 0: uniform routing, mm1 rhs = wm (pre-scaled 1/10) ---
        ps_cc = [psum_s.tile([128, US], f32, name=f"s_ps{bc}") for bc in range(2)]
        for t in range(T):
            g, tl = divmod(t, CPG)
            for bc in range(2):
                nc.tensor.matmul(
                    ps_cc[bc],
                    lhsT=xt_sb[:, t * B + bc * 128 : t * B + bc * 128 + 128],
                    rhs=wm_sb[:, t * US : (t + 1) * US],
                    start=(t == 0),
                    stop=(t == T - 1),
                )
            if tl == CPG - 1 and g < G - 1:
                emit_junk(K_PRE)  # bridge DMA-bound gaps between groups
        cc_sb = work.tile([128, 2 * US], f16, name="cc_sb")
        nc.scalar.copy(out=cc_sb[:, 0:US], in_=ps_cc[0])
        nc.vector.tensor_copy(out=cc_sb[:, US : 2 * US], in_=ps_cc[1])
        jt2 = work.tile([128, JUNK_N], f32, name="jt2")
        nc.vector.tensor_copy(out=jt2, in_=cc_sb[:, 0:JUNK_N])
        emit_junk(K_AR, gate=jt2)
        s_sb = stage_and_reduce(cc_sb, last=False)
        load_xf(jt2)

        # --- iterations 1, 2 ------------------------------------------------
        for it in range(1, NITER):
            v_bf = squash(s_sb)

            # mm2: m[ci, su] = sum_b x[b,ci] v[b,su].  PSUM -> SBUF f16 copies
            # go to Act for the first 14 chunks; the last 4 are deferred to
            # DVE's idle slot after the group-0 tree, so the Act queue (in
            # order!) reaches softmax's exp before all 18 copies drain.
            m16 = big.tile([128, T * US], f16, name="m16")
            deferred = []
            for t in range(T):
                ps = psum_m.tile([128, US], f32, name="m_ps")
                for bc in range(2):
                    nc.tensor.matmul(
                        ps,
                        lhsT=xf_sb[:, bc * KCI + t * 128 : bc * KCI + (t + 1) * 128],
                        rhs=v_bf[:, bc * US : (bc + 1) * US],
                        start=(bc == 0),
                        stop=(bc == 1),
                    )
                if t < N_FRONT:
                    # DVE is idle between squash and q0: it absorbs the
                    # first copies, so m16-g0 completes sooner and Act's
                    # queue reaches softmax exp earlier
                    nc.vector.tensor_copy(out=m16[:, t * US : (t + 1) * US], in_=ps)
                elif t < T - N_DEFER:
                    nc.scalar.copy(out=m16[:, t * US : (t + 1) * US], in_=ps)
                else:
                    deferred.append((t, ps))
                if t == DUMMY_EXP_AT:
                    # dummy exp: pulls the sqrt->exp act-table reload to this
                    # point in the Act queue, off the softmax-g0 chain
                    nc.scalar.activation(
                        out=actwarm, in_=em_sb[:, 0:2], func=AF.Exp
                    )

            ps_cc = [psum_s.tile([128, US], f32, name=f"s_ps{bc}") for bc in range(2)]
            ups = psum_u.tile([128, T * NUM_U], f32, name="u_ps")
            r_sb = work.tile([128, T * NUM_U], f16, name="r_sb")
            ex = work.tile([128, T * NUM_U], f32, name="ex")
            sm = work.tile([128, T], f32, name="sm")
            rsm = work.tile([128, T], f32, name="rsm")
            cij = work.tile([128, T * NUM_U], f16, name="cij")

            # Uneven groups: a small last group shortens the serial tail
            # (q2/tree2/bm2/mm1-g2 gate the cc staging before the collective)
            GS = list(GROUP_SPLIT)
            OFF = [sum(GS[:i]) for i in range(len(GS))]

            def colS(g):
                return slice(OFF[g] * US, (OFF[g] + GS[g]) * US)

            # q = wm * m.  Emission order places q0 and q2 on DVE around the
            # group-0 tree/softmax/bm work; q1 runs on Pool in parallel.
            q_t = [
                big.tile([128, GS[g] * US], f16, name=f"q_{g}") for g in range(G)
            ]

            def emit_q(g):
                if g >= 1:
                    # split: Pool's 0.42-efficiency multiply would gate the
                    # group-1 tree for 3.9us; giving half to DVE pulls the
                    # gate in by ~2us while costing DVE only ~0.5us
                    hn = GS[g] // 2
                    lo = slice(OFF[g] * US, (OFF[g] + hn) * US)
                    hi = slice((OFF[g] + hn) * US, (OFF[g] + GS[g]) * US)
                    nc.gpsimd.tensor_mul(
                        out=q_t[g][:, 0 : hn * US], in0=wm_sb[:, lo], in1=m16[:, lo]
                    )
                    nc.vector.tensor_mul(
                        out=q_t[g][:, hn * US :], in0=wm_sb[:, hi], in1=m16[:, hi]
                    )
                else:
                    nc.vector.tensor_mul(
                        out=q_t[g], in0=wm_sb[:, colS(g)], in1=m16[:, colS(g)]
                    )

            emit_q(0)
            emit_q(1)
            for g in range(G):
                ng = GS[g]
                cS = colS(g)
                uS = slice(OFF[g] * NUM_U, (OFF[g] + ng) * NUM_U)  # u cols
                tS = slice(OFF[g], OFF[g] + ng)
                # r = sum_s q via pairwise-halving adds (TensorTensor runs at
                # 2x for packed f16; TensorReduce is always 1x, so shrink its
                # input first): 32 -> 16 -> 8 s-terms, then reduce.
                qv = q_t[g].rearrange("p (t s u) -> p t s u", t=ng, s=S)
                h1 = big.tile([128, ng * (S // 2) * NUM_U], f16, name="h1")
                h1v = h1.rearrange("p (t s u) -> p t s u", t=ng, s=S // 2)
                nc.vector.tensor_add(
                    out=h1v, in0=qv[:, :, 0 : S // 2, :], in1=qv[:, :, S // 2 : S, :]
                )
                h2 = big.tile([128, ng * (S // 4) * NUM_U], f16, name="h2")
                h2v = h2.rearrange("p (t s u) -> p t s u", t=ng, s=S // 4)
                nc.vector.tensor_add(
                    out=h2v,
                    in0=h1v[:, :, 0 : S // 4, :],
                    in1=h1v[:, :, S // 4 : S // 2, :],
                )
                with nc.allow_low_precision(reason="r in f16: |r|<400, rel 5e-4"):
                    nc.vector.reduce_sum(
                        out=r_sb[:, uS],
                        in_=h2v.transpose([0, 1, 3, 2]),
                        axis=AX.X,
                    )
                # b_ij += EM @ r  (EM carries x10/256); softmax without max-sub
                nc.tensor.matmul(
                    ups[:, uS], lhsT=em_sb, rhs=r_sb[:, uS], start=True, stop=True
                )
                if g == 0:
                    emit_junk_f16(K_MID[0], q_t[0])
                nc.vector.tensor_add(
                    out=bij_sb[:, uS], in0=bij_sb[:, uS], in1=ups[:, uS]
                )
                nc.scalar.activation(out=ex[:, uS], in_=bij_sb[:, uS], func=AF.Exp)
                nc.vector.reduce_sum(
                    out=sm[:, tS],
                    in_=ex[:, uS].rearrange("p (t u) -> p t u", t=ng),
                    axis=AX.X,
                )
                # cij = 10*softmax: the x10 (compensating the 1/10 baked
                # into WM) folds into the normalize via scalar_tensor_tensor
                nc.vector.reciprocal(out=rsm[:, tS], in_=sm[:, tS])
                nc.vector.scalar_tensor_tensor(
                    out=cij[:, uS].rearrange("p (t u) -> p t u", t=ng),
                    in0=ex[:, uS].rearrange("p (t u) -> p t u", t=ng),
                    scalar=10.0,
                    in1=rsm[:, tS].unsqueeze(2).broadcast_to([128, ng, NUM_U]),
                    op0=ALU.mult,
                    op1=ALU.mult,
                )
                bm_g = big.tile([128, ng * US], f16, name="bm_g")
                nc.vector.tensor_mul(
                    out=bm_g.rearrange("p (t s u) -> p t s u", t=ng, s=S),
                    in0=wm_sb[:, cS].rearrange("p (t s u) -> p t s u", t=ng, s=S),
                    in1=cij[:, uS]
                    .rearrange("p (t u) -> p t u", t=ng)
                    .unsqueeze(2)
                    .broadcast_to([128, ng, S, NUM_U]),
                )
                for tl in range(ng):
                    t = OFF[g] + tl
                    for bc in range(2):
                        nc.tensor.matmul(
                            ps_cc[bc],
                            lhsT=xt_sb[:, t * B + bc * 128 : t * B + bc * 128 + 128],
                            rhs=bm_g[:, tl * US : (tl + 1) * US],
                            start=(t == 0),
                            stop=(t == T - 1),
                        )
                if g == 0:
                    # deferred mm2 copies: emitted after the group-0 softmax
                    # chain so DVE prioritizes unblocking bm0/mm1-g0, then
                    # fills its idle with these PSUM reads (they gate q2)
                    for t, ps in deferred:
                        nc.vector.tensor_copy(
                            out=m16[:, t * US : (t + 1) * US], in_=ps
                        )
                    emit_q(2)
                    emit_junk_f16(K_MID[1], q_t[0])
                elif g == 1:
                    emit_junk_f16(K_MID[2], q_t[0])
            cc_sb = work.tile([128, 2 * US], f16, name="cc_sb")
            nc.scalar.copy(out=cc_sb[:, 0:US], in_=ps_cc[0])
            nc.vector.tensor_copy(out=cc_sb[:, US : 2 * US], in_=ps_cc[1])
            last = it == NITER - 1
            if not last:
                jt2 = work.tile([128, JUNK_N], f32, name="jt2")
                nc.vector.tensor_copy(out=jt2, in_=cc_sb[:, 0:JUNK_N])
                emit_junk(K_AR, gate=jt2)
            s_sb = stage_and_reduce(cc_sb, last=last)

    return nc


def _prep_core_inputs(x, W, core, em):
    sl = slice(core * CL, (core + 1) * CL)
    xs = np.ascontiguousarray(x[:, :, sl])  # (B, I, CL)
    ws = np.ascontiguousarray(W[0, sl])     # (CL, U, S, I)
    xt = xs.transpose(2, 1, 0).reshape(T, 128, B)
    xt = np.ascontiguousarray(xt.transpose(1, 0, 2)).reshape(128, T * B)
    xf = xs.transpose(0, 2, 1).reshape(2, 128, KCI)
    xf = np.ascontiguousarray(xf.transpose(1, 0, 2)).reshape(128, 2 * KCI)
    wm = (ws / float(NUM_U)).transpose(0, 3, 2, 1).reshape(T, 128, US)
    wm = np.ascontiguousarray(wm.transpose(1, 0, 2)).reshape(128, T * US)
    return {
        "xt": xt.astype(np.float16),
        "xf": xf.astype(np.float16),
        "wm": wm.astype(np.float16),
        "em": em,
    }


def prep_in_maps(x, W):
    x = np.asarray(x, dtype=np.float32)
    W = np.asarray(W, dtype=np.float32)
    em = (np.kron(np.eye(8, dtype=np.float32), np.ones((16, 16), np.float32))
          * (float(NUM_U) / float(B))).astype(np.float16)
    return [_prep_core_inputs(x, W, core, em) for core in range(NCORES)]


def postprocess(results):
    """Assemble per-core ReduceScatter shards (16 partition rows each) of
    the pre-squash s_j into [128, 640] (col = bc*320 + s*10 + u), apply the
    final squash on the host, then -> (B, U, S, 1)."""
    full = np.concatenate(
        [np.asarray(results[r]["out"], np.float64) for r in range(NCORES)],
        axis=0,
    )
    s = full.reshape(128, 2, S, NUM_U).transpose(1, 0, 3, 2)  # (bc,p,u,s)
    s = s.reshape(B, NUM_U, S)
    m2 = np.sum(s * s, axis=1, keepdims=True)  # norm over the U axis
    v = s * (np.sqrt(m2) / (1.0 + m2))
    return np.ascontiguousarray(v[..., None].astype(np.float32))


def get_program():
    if "nc" not in _CACHE:
        nc = _build_program()
        nc.finalize()
        _CACHE["nc"] = nc
    return _CACHE["nc"]


def kernel(x, W):
    from concourse.bass_utils import run_bass_kernel_spmd

    nc = get_program()
    in_maps = prep_in_maps(x, W)
    res = run_bass_kernel_spmd(nc, in_maps, list(range(NCORES)))
    return postprocess(res.results)
